# revision 2
# baseline (speedup 1.0000x reference)
"""LIFSpike Trainium2 kernel (Bass/Tile), SPMD over 8 NeuronCores.

Reference semantics (T=4, tau=2, vth=1, vreset=0, decay_input=False,
detach_reset, hard reset):
    xs = x.reshape(T, B//T, C, H, W); v0 = 0
    h_t = 0.5 * v_t + x_t
    s_t = (h_t >= 1.0)
    v_{t+1} = h_t * (h_t < 1.0)
    out = s.reshape(B, C, H, W)

Kernel-side reformulation (exact in fp32 -- all rescalings are powers
of two, which commute with fp rounding):
    r_t := 2^t * h_t,  host supplies x'_t = 2^t * x_t
    r_0     = x'_0                        (DMA load)
    s_t     = (r_t >= 2^t)                (DVE tensor_scalar is_ge -> u8)
    q_t     = (r_t < 2^t) * r_t           (DVE STT, in-place)
    r_{t+1} = q_t + x'_{t+1}              (DVE tensor_tensor add)

Design (measured on HW via K-slope benching, see transcript):
  - data parallel over the per-timestep batch dim: 4 chains/core
  - software pipeline over NCH=4 free-dim chunks of 1 MiB; the serial
    T-chain of each chunk overlaps with the DMA/compute of the others
  - spike output stored as uint8 -> 1/4 the store bytes of f32
  - x' loads are PLAIN HWDGE DMAs + DVE adds: the accum-DMA (CCE RMW)
    alternative costs 2x SBUF-AXI port bytes and measured ~9 us/iter
    slower; gpsimd tensor_tensor adds measured far slower still
  - loads ride the SP HWDGE ring (nc.sync), stores the ACT ring
    (nc.scalar), so store waits never block next-iteration loads
  - x' chunk loads are emitted one timestep-section ahead
"""

import numpy as np

T = 4
BP = 32               # B // T
NCORES = 8
BPC = BP // NCORES    # chains per core = 4
SLICE = 256 * 32 * 32  # elements per (t, b) slice = 262144
P = 128
FREE_B = SLICE // P   # 2048
FREE_T = BPC * FREE_B  # 8192 free elements per timestep per core
FREE = T * FREE_T     # 32768

NCH = 4               # pipeline chunks
N_DVE_ADDS = 12       # of the (T-1)*NCH adds, how many on DVE (rest accum)
STORE_ENGINE = "scalar"

_cache = {}


def build_program(reps=1, nch=NCH, n_dve_adds=N_DVE_ADDS,
                  store_engine=STORE_ENGINE, bufs=2, xin_bufs=2):
    import concourse.bass as bass
    import concourse.tile as tile
    from concourse import bacc, mybir

    Alu = mybir.AluOpType
    f32 = mybir.dt.float32
    u8 = mybir.dt.uint8
    F = FREE_T // nch

    nc = bacc.Bacc(debug=False)
    x = nc.dram_tensor("x", [P, FREE], f32, kind="ExternalInput").ap()
    s = nc.dram_tensor("s", [P, FREE], u8, kind="ExternalOutput").ap()

    st_eng = {"sync": nc.sync, "scalar": nc.scalar}[store_engine]

    n_adds = (T - 1) * nch
    # 'd' -> DVE tensor_tensor add of a plain-loaded x' tile;
    # 'a' -> gpsimd accum-DMA (only used when n_dve_adds < n_adds)
    dve_slots = {(k * n_adds) // max(1, n_dve_adds)
                 for k in range(n_dve_adds)}
    add_kind = ["d" if i in dve_slots else "a" for i in range(n_adds)]

    with tile.TileContext(nc) as tc:
        # tiles are ring-buffered per (pool, tag); tag defaults to the
        # tile name, so each chunk gets its own `bufs`-deep ring
        with (
            tc.tile_pool(name="state", bufs=bufs) as rpool,
            tc.tile_pool(name="sout", bufs=bufs) as opool,
            tc.tile_pool(name="xin", bufs=xin_bufs) as xpool,
        ):
            for rep in range(reps):
                r = [rpool.tile([P, F], f32, name=f"r{c}")
                     for c in range(nch)]
                o = [opool.tile([P, T * F], u8, name=f"o{c}")
                     for c in range(nch)]
                xt = {}

                def emit_xin_loads(tr):
                    # x' loads for transition tr, emitted one section early
                    if tr > T - 2:
                        return
                    base = (tr + 1) * FREE_T
                    for c in range(nch):
                        if add_kind[tr * nch + c] == "d":
                            xtile = xpool.tile([P, F], f32, name=f"x{c}")
                            nc.sync.dma_start(
                                xtile[:],
                                x[:, base + c * F:base + (c + 1) * F],
                            )
                            xt[(tr, c)] = xtile

                for c in range(nch):
                    nc.sync.dma_start(r[c][:], x[:, c * F:(c + 1) * F])
                emit_xin_loads(0)
                for t in range(T):
                    th = float(1 << t)
                    emit_xin_loads(t + 1)
                    for c in range(nch):
                        nc.vector.tensor_scalar(
                            o[c][:, t * F:(t + 1) * F], r[c][:], th, None,
                            Alu.is_ge,
                        )
                        if t < T - 1:
                            nc.vector.scalar_tensor_tensor(
                                r[c][:], r[c][:], th, r[c][:],
                                Alu.is_lt, Alu.mult,
                            )
                            if add_kind[t * nch + c] == "d":
                                nc.vector.tensor_tensor(
                                    r[c][:], r[c][:], xt[(t, c)][:], Alu.add
                                )
                    if t < T - 1:
                        base = (t + 1) * FREE_T
                        for c in range(nch):
                            if add_kind[t * nch + c] == "a":
                                nc.gpsimd.dma_start(
                                    r[c][:],
                                    x[:, base + c * F:base + (c + 1) * F],
                                    accum_op=Alu.add,
                                )
                    else:
                        for c in range(nch):
                            st_eng.dma_start(
                                s[:, c * T * F:(c + 1) * T * F], o[c][:]
                            )
    nc.compile()
    return nc


def _shard(x):
    # x: (128, 256, 32, 32) f32 -> 8 per-core [128, 32768] f32 arrays,
    # timestep t pre-scaled by 2^t (exact in fp32); layout
    # x_core[p, t*8192 + b*2048 + j] = 2^t * x[t*32 + core*4 + b, flat]
    xr = np.ascontiguousarray(x).reshape(T, BP, SLICE)
    tscale = (2.0 ** np.arange(T, dtype=np.float32)).astype(np.float32)
    shards = []
    for k in range(NCORES):
        xk = xr[:, k * BPC:(k + 1) * BPC, :].reshape(T, BPC, P, FREE_B)
        xk = xk * tscale[:, None, None, None]
        xk = xk.transpose(2, 0, 1, 3).reshape(P, FREE)
        shards.append(np.asarray(xk, dtype=np.float32))
    return shards


def _unshard(parts, nch=NCH):
    # parts: 8 per-core [128, 32768] u8 arrays, chunk-major layout
    # s[p, c*T*F + t*F + j] -> (128,256,32,32) f32
    F = FREE_T // nch
    cpb = nch // BPC  # chunks per chain
    out = np.empty((T, BP, SLICE), dtype=np.float32)
    for k, sk in enumerate(parts):
        sk = np.asarray(sk).astype(np.float32).reshape(P, BPC, cpb, T, F)
        out[:, k * BPC:(k + 1) * BPC, :] = (
            sk.transpose(3, 1, 0, 2, 4).reshape(T, BPC, SLICE)
        )
    return out.reshape(T * BP, 256, 32, 32)


def kernel(x):
    from concourse.bass_utils import run_bass_kernel_spmd

    if "nc" not in _cache:
        _cache["nc"] = build_program()
    nc = _cache["nc"]

    shards = _shard(np.asarray(x, dtype=np.float32))
    in_maps = [{"x": sk} for sk in shards]
    res = run_bass_kernel_spmd(nc, in_maps, list(range(NCORES)))
    return _unshard([res.results[k]["s"] for k in range(NCORES)])
